# revision 67
# baseline (speedup 1.0000x reference)
"""Trainium2 Bass kernel for single-head causal attention.

Problem: x[B=4,T=2048,C=1024] -> q,k,v = x@Wq/Wk/Wv [T,64] -> causal softmax(q k^T/sqrt(C)) @ v.

Sharding: 8 cores = 4 batches x 2 query-interleavings. Core r of a batch owns
the 8 INTERLEAVED query blocks g === r (mod 2) (128 rows each), which balances
causal work across the pair (each core gets ~half the attention area).

SPMD-uniform trick: the time axis of each core's x^T copy is permuted so the
core's OWN blocks come first in DESCENDING global order (columns 0-1023), the
other 8 blocks after (descending). Then the block-causal structure is
identical on every core and every S k-slot covers a PREFIX of the query axis:
  - k-slot t=0..7  (own block (14+r)-2t): S over q cols [0, 128(t+1)); the
    trailing 128x128 block is the diagonal -> multiplied by a triangular mask.
  - k-slot t=8..15 (other block (15-r)-2(t-8)): S over q cols [0, 128(t-7));
    the trailing block differs only by DATA: all-ones (r=1: k-block just
    below the diagonal -> keep) or all-zeros (r=0: just above -> drop).
Prefix ranges mean S/exp for slots 0..3 need only the first quarter of x, so
the scalar-engine exp pipeline (the S-phase bottleneck) starts while x is
still streaming in.

Softmax normalization is fused into the AV matmul by appending a ones column
to V (output row 64 = sum of exp); division happens host-side on gather.

All matmuls stream bf16. x is loaded as 16 separate 256KB DMAs (the ~0.65us
per-trigger cost paces the queue so transfers pipeline instead of splitting
HBM bandwidth round-robin). Projections run chunk-major per time-quarter. V
is transposed to natural layout with four batched DMA-xbar transposes.
"""

import numpy as np
import ml_dtypes

B, T, C, H = 4, 2048, 1024, 64
TQ = 1024          # queries per core
NT = 2048          # kv length per core
NCH = C // 128     # 8 contraction chunks
NKT = NT // 128    # 16 k-slots
SCALE = 1.0 / 32.0  # 1/sqrt(C)
VSTRIDE = 80       # bf16 cols per v' slot (64 v + 1 ones + pad, 32B-aligned)

_prog_cache = {}


def _build_program():
    import concourse.mybir as mybir
    from concourse import bacc
    from concourse.tile import TileContext

    fp32 = mybir.dt.float32
    bf16 = mybir.dt.bfloat16
    Exp = mybir.ActivationFunctionType.Exp
    Copy = mybir.ActivationFunctionType.Copy

    nc = bacc.Bacc("TRN2", target_bir_lowering=False, debug=False)

    # xt is pre-tiled host-side: row-block g=4*tb+u is one [128,1024] SBUF
    # tile (time-quarter tb, C-chunk pair 2u|2u+1), DRAM-contiguous.
    xt_d = nc.dram_tensor("xt", [NT, C], bf16, kind="ExternalInput")
    wcat_d = nc.dram_tensor("wcat", [C, 192], bf16, kind="ExternalInput")
    trig_d = nc.dram_tensor("trig", [128, 256], bf16, kind="ExternalInput")
    out_d = nc.dram_tensor("outT", [H + 1, TQ], bf16, kind="ExternalOutput")
    scr_d = nc.dram_tensor("scr", [64, 64], bf16, kind="Internal")

    with TileContext(nc) as tc:
        with (
            tc.tile_pool(name="xtp", bufs=1) as xt_pool,
            tc.tile_pool(name="cst", bufs=1) as cst,
            tc.tile_pool(name="prj", bufs=1) as prj,
            tc.tile_pool(name="ptp", bufs=16) as ptp,
            tc.tile_pool(name="pqk", bufs=2, space="PSUM") as pqk,
            tc.tile_pool(name="pvo", bufs=2, space="PSUM") as pvo,
            tc.tile_pool(name="pss", bufs=2, space="PSUM") as pss,
        ):
            # const DMAs on the scalar queue (idle until exp starts)
            wcat_sb = cst.tile([128, NCH, 192], bf16, tag="wcat")
            nc.scalar.dma_start(out=wcat_sb[:], in_=wcat_d.rearrange("(o p) m -> p o m", p=128))
            trig_sb = cst.tile([128, 256], bf16, tag="trig")
            nc.scalar.dma_start(out=trig_sb[:], in_=trig_d[:])

            # x^T time-quarters, 4 chunk-pair DMAs each, on the sync queue.
            # Tiles rotate through 4 pool slots: DMA g+4 WAR-waits until the
            # proj matmuls consumed tile g, capping in-flight transfers so
            # arrivals pipeline instead of splitting HBM bandwidth 8+ ways.
            xtiles = [xt_pool.tile([128, 1024], bf16, tag=f"x{g}", bufs=1,
                                   name=f"x{g}") for g in range(16)]
            xd = [xtiles[4 * tb: 4 * tb + 4] for tb in range(4)]
            for g in range(16):
                nc.sync.dma_start(out=xtiles[g][:],
                                  in_=xt_d[128 * g: 128 * (g + 1), :])

            def xch(tb, c):
                """rhs AP [128, 512] for time-quarter tb, contraction chunk c."""
                return xd[tb][c // 2][:, 512 * (c % 2): 512 * (c % 2) + 512]

            # persistent projection outputs (bf16 so S/AV stream at full rate)
            qT_sb = prj.tile([64, TQ], bf16, tag="qT")
            kT_sb = prj.tile([64, TQ], bf16, tag="kT")      # own-half k
            kvT_sb = prj.tile([128, TQ], bf16, tag="kvT")   # rows 0-63 k-oth, 64-127 v-oth
            vT_sb = prj.tile([64, TQ], bf16, tag="vT")      # v-own (transposed)
            vp_sb = prj.tile([128, NKT * VSTRIDE], bf16, tag="vp")
            o_sb = prj.tile([H + 1, TQ], bf16, tag="osb")

            vp3 = vp_sb.rearrange("p (t c) -> p t c", c=VSTRIDE)
            nc.gpsimd.memset(vp3[:, :, 64:65], 1.0)

            # ---- PE warmup on memset data: keep the HAM activity window
            # busy from engine-init through the first x arrivals ----
            ws_sb = prj.tile([128, 256], bf16, tag="ws")
            wsc_sb = prj.tile([64, 64], bf16, tag="wsc")
            nc.vector.memset(ws_sb[:], 0.125)
            w_ps = pss.tile([128, 256], fp32, tag="s")
            for _ in range(16):
                nc.tensor.matmul(w_ps[:], ws_sb[:, 0:128], ws_sb[:],
                                 start=True, stop=True)
            nc.vector.tensor_copy(out=wsc_sb[:], in_=w_ps[0:64, 0:64])

            # ---- projection passes (chunk-major within a time-quarter) ----
            def emit_proj(ps, lhs_lo, lhs_hi, tb):
                for c in range(NCH):
                    nc.tensor.matmul(
                        ps[:], wcat_sb[:, c, lhs_lo:lhs_hi], xch(tb, c),
                        start=(c == 0), stop=(c == NCH - 1),
                    )

            def emit_proj_qkv(qk_ps, v_ps, tb):
                # same-psum matmul runs back to back (bank-alternating costs
                # ~60ns/MM extra); v trails by a chunk so it never stalls
                for c in range(NCH):
                    nc.tensor.matmul(
                        qk_ps[:], wcat_sb[:, c, 0:128], xch(tb, c),
                        start=(c == 0), stop=(c == NCH - 1),
                    )
                for c in range(NCH):
                    nc.tensor.matmul(
                        v_ps[:], wcat_sb[:, c, 128:192], xch(tb, c),
                        start=(c == 0), stop=(c == NCH - 1),
                    )

            # ---- S / exp / AV phase ----
            pt_tiles = {}

            def emit_S(t):
                tp = t % 8 if t < 8 else 15 - t
                hi = 128 * (tp + 1)
                kt = (kT_sb[:, 128 * tp: 128 * (tp + 1)] if t < 8
                      else kvT_sb[0:64, 128 * (t - 8): 128 * (t - 7)])
                s = pss.tile([128, 1024], fp32, tag="s")
                nc.tensor.matmul(s[:, 0:min(hi, 512)], kt,
                                 qT_sb[:, 0:min(hi, 512)],
                                 start=True, stop=True)
                if hi > 512:
                    nc.tensor.matmul(s[:, 512:hi], kt, qT_sb[:, 512:hi],
                                     start=True, stop=True)
                pt = ptp.tile([128, 1024], bf16, tag="pt")
                nc.scalar.activation(pt[:, 0:hi], s[:, 0:hi], Exp, scale=SCALE)
                pt_tiles[t] = pt

            def emit_mask(t):
                tp = t % 8 if t < 8 else 15 - t
                hi = 128 * (tp + 1)
                msk = trig_sb[:, 0:128] if t < 8 else trig_sb[:, 128:256]
                pt = pt_tiles[t]
                nc.vector.tensor_mul(pt[:, 128 * tp: hi], pt[:, 128 * tp: hi], msk)

            o_ps = []

            def emit_AV(t):
                tp = t % 8 if t < 8 else 15 - t
                hi = 128 * (tp + 1)
                pt = pt_tiles.pop(t)
                nc.tensor.matmul(
                    o_ps[0][:, 0:min(hi, 512)],
                    vp3[:, t, 0:65], pt[:, 0:min(hi, 512)],
                    start=(t == 0), stop=(t == 15),
                    skip_group_check=True,
                )
                if hi > 512:
                    nc.tensor.matmul(
                        o_ps[1][:, 0:hi - 512],
                        vp3[:, t, 0:65], pt[:, 512:hi],
                        start=(t == 4), stop=(t == 11),
                        skip_group_check=True,
                    )

            # ================= program order =================
            # pass 1-A: own cols [0:512) -> q[0:512), k-own 0..3, v-own-A
            qk_psA = pqk.tile([128, 512], fp32, tag="qk")
            v_psA = pvo.tile([64, 512], fp32, tag="vo", padded_shape=[128, 512])
            emit_proj_qkv(qk_psA, v_psA, 0)
            nc.vector.tensor_copy(out=qT_sb[:, 0:512], in_=qk_psA[0:64, :])
            nc.scalar.activation(kT_sb[:, 0:512], qk_psA[64:128, :], Copy)
            nc.vector.tensor_copy(out=vT_sb[:, 0:512], in_=v_psA[:])

            emit_S(0)
            emit_S(1)
            emit_S(2)
            emit_S(3)
            emit_mask(0)
            emit_mask(1)
            emit_mask(2)
            emit_mask(3)

            # pass 1-B: own cols [512:1024)
            qk_psB = pqk.tile([128, 512], fp32, tag="qk")
            v_psB = pvo.tile([64, 512], fp32, tag="vo", padded_shape=[128, 512])
            emit_proj_qkv(qk_psB, v_psB, 1)
            nc.vector.tensor_copy(out=qT_sb[:, 512:1024], in_=qk_psB[0:64, :])
            nc.scalar.activation(kT_sb[:, 512:1024], qk_psB[64:128, :], Copy)
            nc.vector.tensor_copy(out=vT_sb[:, 512:1024], in_=v_psB[:])

            o_ps.append(pvo.tile([H + 1, 512], fp32, tag="vo", name="o0",
                                 padded_shape=[128, 512]))
            o_ps.append(pvo.tile([H + 1, 512], fp32, tag="vo", name="o1",
                                 padded_shape=[128, 512]))

            # all own-half v -> natural layout, one batched xbar transpose
            nc.sync.dma_start_transpose(out=vp3[:, 0:8, 0:64], in_=vT_sb[:])

            emit_S(4)
            emit_S(5)
            emit_S(6)
            emit_S(7)

            # pass 2-A: other cols [0:512) -> k-oth 0..3 + v-oth (packed)
            kv_psA = pqk.tile([128, 512], fp32, tag="qk")
            emit_proj(kv_psA, 64, 192, 2)
            nc.vector.tensor_copy(out=kvT_sb[:, 0:512], in_=kv_psA[:])
            nc.sync.dma_start_transpose(out=vp3[:, 8:12, 0:64],
                                        in_=kvT_sb[64:128, 0:512])

            emit_mask(4)
            emit_mask(5)
            emit_mask(6)
            emit_mask(7)
            emit_S(8)
            emit_mask(8)
            emit_S(9)
            emit_mask(9)
            emit_S(10)
            emit_mask(10)
            emit_S(11)
            emit_mask(11)
            emit_AV(0)
            emit_AV(1)
            emit_AV(2)
            emit_AV(3)
            emit_AV(4)
            emit_AV(5)
            emit_AV(6)
            emit_AV(7)
            emit_AV(8)
            emit_AV(9)
            emit_AV(10)
            emit_AV(11)
            # o1 is complete at AV(11): drain it under the remaining tail
            nc.vector.tensor_copy(out=o_sb[:, 512:1024], in_=o_ps[1][:])
            nc.sync.dma_start(out=out_d[:, 512:1024], in_=o_sb[:, 512:1024])

            # pass 2-B: other cols [512:1024)
            kv_psB = pqk.tile([128, 512], fp32, tag="qk")
            emit_proj(kv_psB, 64, 192, 3)
            nc.vector.tensor_copy(out=kvT_sb[:, 512:1024], in_=kv_psB[:])
            nc.sync.dma_start_transpose(out=vp3[:, 12:16, 0:64],
                                        in_=kvT_sb[64:128, 512:1024])

            # ascending other-half order means slots 12-15 shrink (512..128
            # cols), so the last-arriving x quarter carries the SMALLEST exps
            emit_S(12)
            emit_mask(12)
            emit_S(13)
            emit_mask(13)
            emit_S(14)
            emit_mask(14)
            emit_S(15)
            emit_mask(15)
            emit_AV(12)
            emit_AV(13)
            emit_AV(14)
            emit_AV(15)
            nc.scalar.activation(o_sb[:, 0:512], o_ps[0][:], Copy)
            nc.sync.dma_start(out=out_d[:, 0:512], in_=o_sb[:, 0:512])

    nc.finalize()
    return nc


def _get_program():
    if "nc" not in _prog_cache:
        _prog_cache["nc"] = _build_program()
    return _prog_cache["nc"]


def make_in_maps(x, Wq, Wk, Wv):
    bf16 = ml_dtypes.bfloat16
    wcat = np.concatenate([Wq, Wk, Wv], axis=1).astype(bf16)  # [C, 192]
    tri = np.triu(np.ones((128, 128), np.float32))  # tri[k,q]=1 iff q>=k
    in_maps = []
    for core in range(8):
        b, r = core // 2, core % 2
        xb = np.asarray(x[b]).reshape(16, 128, C)
        own_blocks = [(14 + r) - 2 * i for i in range(8)]
        oth_blocks = [(1 - r) + 2 * m for m in range(8)]
        own = xb[own_blocks].reshape(TQ, C)
        other = xb[oth_blocks].reshape(TQ, C)
        xtT = np.concatenate([own, other], axis=0).T  # [C, 2048]
        # pre-tile for the DMA: row-block g=4*tb+u of the [2048,1024] DRAM
        # tensor holds (time-quarter tb) x (chunk pair 2u|2u+1)
        xt = (xtT.reshape(4, 2, 128, 4, 512)
              .transpose(3, 0, 2, 1, 4).reshape(NT, C))
        gate = np.full((128, 128), float(r), np.float32)
        trig = np.concatenate([tri, gate], axis=1).astype(bf16)  # [128, 256]
        in_maps.append({
            "xt": np.ascontiguousarray(xt).astype(bf16),
            "wcat": wcat,
            "trig": trig,
        })
    return in_maps


def postprocess(results):
    out = np.empty((B, T, H), np.float32)
    for core in range(8):
        b, r = core // 2, core % 2
        oT = np.asarray(results[core]["outT"], np.float32)  # [65, 1024]
        o = (oT[:H] / oT[H:H + 1]).T  # [1024, 64] local q order
        for i in range(8):
            g = (14 + r) - 2 * i
            out[b, 128 * g: 128 * (g + 1)] = o[128 * i: 128 * (i + 1)]
    return out


def kernel(x, mask, Wq, Wk, Wv, _trace=False, _tracedir=None):
    from concourse import bass_utils

    nc = _get_program()
    in_maps = make_in_maps(np.asarray(x, np.float32), np.asarray(Wq, np.float32),
                           np.asarray(Wk, np.float32), np.asarray(Wv, np.float32))
    res = bass_utils.run_bass_kernel_spmd(
        nc, in_maps, core_ids=list(range(8)),
        trace=_trace, tmpdir=_tracedir,
    )
    out = postprocess(res.results)
    if _trace:
        return out, res
    return out


# revision 69
# speedup vs baseline: 1.0948x; 1.0948x over previous
"""Trainium2 Bass kernel for single-head causal attention.

Problem: x[B=4,T=2048,C=1024] -> q,k,v = x@Wq/Wk/Wv [T,64] -> causal softmax(q k^T/sqrt(C)) @ v.

Sharding: 8 cores = 4 batches x 2 query-interleavings. Core r of a batch owns
the 8 INTERLEAVED query blocks g === r (mod 2) (128 rows each), which balances
causal work across the pair (each core gets ~half the attention area).

SPMD-uniform trick: the time axis of each core's x^T copy is permuted so the
core's OWN blocks come first in DESCENDING global order (columns 0-1023), the
other 8 blocks after (descending). Then the block-causal structure is
identical on every core and every S k-slot covers a PREFIX of the query axis:
  - k-slot t=0..7  (own block (14+r)-2t): S over q cols [0, 128(t+1)); the
    trailing 128x128 block is the diagonal -> multiplied by a triangular mask.
  - k-slot t=8..15 (other block (15-r)-2(t-8)): S over q cols [0, 128(t-7));
    the trailing block differs only by DATA: all-ones (r=1: k-block just
    below the diagonal -> keep) or all-zeros (r=0: just above -> drop).
Prefix ranges mean S/exp for slots 0..3 need only the first quarter of x, so
the scalar-engine exp pipeline (the S-phase bottleneck) starts while x is
still streaming in.

Softmax normalization is fused into the AV matmul by appending a ones column
to V (output row 64 = sum of exp); division happens host-side on gather.

All matmuls stream bf16. x is loaded as 16 separate 256KB DMAs (the ~0.65us
per-trigger cost paces the queue so transfers pipeline instead of splitting
HBM bandwidth round-robin). Projections run chunk-major per time-quarter. V
is transposed to natural layout with four batched DMA-xbar transposes.
"""

import numpy as np
import ml_dtypes

B, T, C, H = 4, 2048, 1024, 64
TQ = 1024          # queries per core
NT = 2048          # kv length per core
NCH = C // 128     # 8 contraction chunks
NKT = NT // 128    # 16 k-slots
SCALE = 1.0 / 32.0  # 1/sqrt(C)
VSTRIDE = 80       # bf16 cols per v' slot (64 v + 1 ones + pad, 32B-aligned)

_prog_cache = {}


def _build_program():
    import concourse.mybir as mybir
    from concourse import bacc
    from concourse.tile import TileContext

    fp32 = mybir.dt.float32
    bf16 = mybir.dt.bfloat16
    Exp = mybir.ActivationFunctionType.Exp
    Copy = mybir.ActivationFunctionType.Copy

    nc = bacc.Bacc("TRN2", target_bir_lowering=False, debug=False)

    # xt is pre-tiled host-side: row-block g=4*tb+u is one [128,1024] SBUF
    # tile (time-quarter tb, C-chunk pair 2u|2u+1), DRAM-contiguous.
    xt_d = nc.dram_tensor("xt", [NT, C], bf16, kind="ExternalInput")
    wcat_d = nc.dram_tensor("wcat", [C, 192], bf16, kind="ExternalInput")
    trig_d = nc.dram_tensor("trig", [128, 256], bf16, kind="ExternalInput")
    out_d = nc.dram_tensor("outT", [H + 1, TQ], bf16, kind="ExternalOutput")
    scr_d = nc.dram_tensor("scr", [64, 64], bf16, kind="Internal")

    with TileContext(nc) as tc:
        with (
            tc.tile_pool(name="xtp", bufs=1) as xt_pool,
            tc.tile_pool(name="cst", bufs=1) as cst,
            tc.tile_pool(name="prj", bufs=1) as prj,
            tc.tile_pool(name="ptp", bufs=16) as ptp,
            tc.tile_pool(name="pqk", bufs=2, space="PSUM") as pqk,
            tc.tile_pool(name="pvo", bufs=2, space="PSUM") as pvo,
            tc.tile_pool(name="pss", bufs=2, space="PSUM") as pss,
        ):
            # const DMAs on the scalar queue (idle until exp starts)
            wcat_sb = cst.tile([128, NCH, 192], bf16, tag="wcat")
            nc.scalar.dma_start(out=wcat_sb[:], in_=wcat_d.rearrange("(o p) m -> p o m", p=128))
            trig_sb = cst.tile([128, 256], bf16, tag="trig")
            nc.scalar.dma_start(out=trig_sb[:], in_=trig_d[:])

            # x^T time-quarters, 4 chunk-pair DMAs each, on the sync queue.
            # Tiles rotate through 4 pool slots: DMA g+4 WAR-waits until the
            # proj matmuls consumed tile g, capping in-flight transfers so
            # arrivals pipeline instead of splitting HBM bandwidth 8+ ways.
            xtiles = [xt_pool.tile([128, 1024], bf16, tag=f"x{g}", bufs=1,
                                   name=f"x{g}") for g in range(16)]
            xd = [xtiles[4 * tb: 4 * tb + 4] for tb in range(4)]
            for g in range(16):
                nc.sync.dma_start(out=xtiles[g][:],
                                  in_=xt_d[128 * g: 128 * (g + 1), :])

            def xch(tb, c):
                """rhs AP [128, 512] for time-quarter tb, contraction chunk c."""
                return xd[tb][c // 2][:, 512 * (c % 2): 512 * (c % 2) + 512]

            # persistent projection outputs (bf16 so S/AV stream at full rate)
            qT_sb = prj.tile([64, TQ], bf16, tag="qT")
            kT_sb = prj.tile([64, TQ], bf16, tag="kT")      # own-half k
            kvT_sb = prj.tile([128, TQ], bf16, tag="kvT")   # rows 0-63 k-oth, 64-127 v-oth
            vT_sb = prj.tile([64, TQ], bf16, tag="vT")      # v-own (transposed)
            vp_sb = prj.tile([128, NKT * VSTRIDE], bf16, tag="vp")
            o_sb = prj.tile([H + 1, TQ], bf16, tag="osb")

            vp3 = vp_sb.rearrange("p (t c) -> p t c", c=VSTRIDE)
            nc.gpsimd.memset(vp3[:, :, 64:65], 1.0)

            # ---- PE warmup on memset data: keep the HAM activity window
            # busy from engine-init through the first x arrivals ----
            ws_sb = prj.tile([128, 256], bf16, tag="ws")
            wsc_sb = prj.tile([64, 64], bf16, tag="wsc")
            nc.vector.memset(ws_sb[:], 0.125)
            w_ps = pss.tile([128, 256], fp32, tag="s")
            for _ in range(16):
                nc.tensor.matmul(w_ps[:], ws_sb[:, 0:128], ws_sb[:],
                                 start=True, stop=True)
            nc.vector.tensor_copy(out=wsc_sb[:], in_=w_ps[0:64, 0:64])

            # ---- projection passes (chunk-major within a time-quarter) ----
            def emit_proj(ps, lhs_lo, lhs_hi, tb):
                for c in range(NCH):
                    nc.tensor.matmul(
                        ps[:], wcat_sb[:, c, lhs_lo:lhs_hi], xch(tb, c),
                        start=(c == 0), stop=(c == NCH - 1),
                    )

            def emit_proj_qkv(qk_ps, v_ps, tb):
                # same-psum matmul runs back to back (bank-alternating costs
                # ~60ns/MM extra); v trails by a chunk so it never stalls
                for c in range(NCH):
                    nc.tensor.matmul(
                        qk_ps[:], wcat_sb[:, c, 0:128], xch(tb, c),
                        start=(c == 0), stop=(c == NCH - 1),
                    )
                for c in range(NCH):
                    nc.tensor.matmul(
                        v_ps[:], wcat_sb[:, c, 128:192], xch(tb, c),
                        start=(c == 0), stop=(c == NCH - 1),
                    )

            # ---- S / exp / AV phase ----
            pt_tiles = {}

            def emit_S(t):
                tp = t % 8 if t < 8 else 15 - t
                hi = 128 * (tp + 1)
                kt = (kT_sb[:, 128 * tp: 128 * (tp + 1)] if t < 8
                      else kvT_sb[0:64, 128 * (t - 8): 128 * (t - 7)])
                s = pss.tile([128, 1024], fp32, tag="s")
                nc.tensor.matmul(s[:, 0:min(hi, 512)], kt,
                                 qT_sb[:, 0:min(hi, 512)],
                                 start=True, stop=True)
                if hi > 512:
                    nc.tensor.matmul(s[:, 512:hi], kt, qT_sb[:, 512:hi],
                                     start=True, stop=True)
                pt = ptp.tile([128, 1024], bf16, tag="pt")
                nc.scalar.activation(pt[:, 0:hi], s[:, 0:hi], Exp, scale=SCALE)
                pt_tiles[t] = pt

            def emit_mask(t):
                tp = t % 8 if t < 8 else 15 - t
                hi = 128 * (tp + 1)
                msk = trig_sb[:, 0:128] if t < 8 else trig_sb[:, 128:256]
                pt = pt_tiles[t]
                nc.vector.tensor_mul(pt[:, 128 * tp: hi], pt[:, 128 * tp: hi], msk)

            o_ps = []

            def emit_AV(t):
                tp = t % 8 if t < 8 else 15 - t
                hi = 128 * (tp + 1)
                pt = pt_tiles.pop(t)
                nc.tensor.matmul(
                    o_ps[0][:, 0:min(hi, 512)],
                    vp3[:, t, 0:65], pt[:, 0:min(hi, 512)],
                    start=(t == 0), stop=(t == 15),
                    skip_group_check=True,
                )
                if hi > 512:
                    nc.tensor.matmul(
                        o_ps[1][:, 0:hi - 512],
                        vp3[:, t, 0:65], pt[:, 512:hi],
                        start=(t == 4), stop=(t == 11),
                        skip_group_check=True,
                    )

            # ================= program order =================
            # pass 1-A: own cols [0:512) -> q[0:512), k-own 0..3, v-own-A
            qk_psA = pqk.tile([128, 512], fp32, tag="qk")
            v_psA = pvo.tile([64, 512], fp32, tag="vo", padded_shape=[128, 512])
            emit_proj_qkv(qk_psA, v_psA, 0)
            # halves: S(0)/S(1) only need the first 256 cols of q,k
            nc.vector.tensor_copy(out=qT_sb[:, 0:256], in_=qk_psA[0:64, 0:256])
            nc.scalar.activation(kT_sb[:, 0:256], qk_psA[64:128, 0:256], Copy)
            nc.vector.tensor_copy(out=qT_sb[:, 256:512], in_=qk_psA[0:64, 256:512])
            nc.scalar.activation(kT_sb[:, 256:512], qk_psA[64:128, 256:512], Copy)
            nc.vector.tensor_copy(out=vT_sb[:, 0:512], in_=v_psA[:])

            emit_S(0)
            emit_S(1)
            emit_S(2)
            emit_S(3)
            emit_mask(0)
            emit_mask(1)
            emit_mask(2)
            emit_mask(3)

            # pass 1-B: own cols [512:1024)
            qk_psB = pqk.tile([128, 512], fp32, tag="qk")
            v_psB = pvo.tile([64, 512], fp32, tag="vo", padded_shape=[128, 512])
            emit_proj_qkv(qk_psB, v_psB, 1)
            nc.vector.tensor_copy(out=qT_sb[:, 512:1024], in_=qk_psB[0:64, :])
            nc.scalar.activation(kT_sb[:, 512:1024], qk_psB[64:128, :], Copy)
            nc.vector.tensor_copy(out=vT_sb[:, 512:1024], in_=v_psB[:])

            o_ps.append(pvo.tile([H + 1, 512], fp32, tag="vo", name="o0",
                                 padded_shape=[128, 512]))
            o_ps.append(pvo.tile([H + 1, 512], fp32, tag="vo", name="o1",
                                 padded_shape=[128, 512]))

            # all own-half v -> natural layout, one batched xbar transpose
            nc.sync.dma_start_transpose(out=vp3[:, 0:8, 0:64], in_=vT_sb[:])

            emit_S(4)
            emit_S(5)
            emit_S(6)
            emit_S(7)

            # pass 2-A: other cols [0:512) -> k-oth 0..3 + v-oth (packed)
            kv_psA = pqk.tile([128, 512], fp32, tag="qk")
            emit_proj(kv_psA, 64, 192, 2)
            nc.vector.tensor_copy(out=kvT_sb[:, 0:512], in_=kv_psA[:])

            emit_mask(4)
            emit_mask(5)
            emit_mask(6)
            emit_mask(7)
            emit_S(8)
            emit_mask(8)
            emit_S(9)
            emit_mask(9)
            emit_S(10)
            emit_mask(10)
            emit_S(11)
            emit_mask(11)
            emit_AV(0)
            emit_AV(1)
            emit_AV(2)
            emit_AV(3)

            # pass 2-B: other cols [512:1024)
            kv_psB = pqk.tile([128, 512], fp32, tag="qk")
            emit_proj(kv_psB, 64, 192, 3)
            nc.vector.tensor_copy(out=kvT_sb[:, 512:1024], in_=kv_psB[:])
            # all other-half v -> natural layout
            nc.sync.dma_start_transpose(out=vp3[:, 8:16, 0:64],
                                        in_=kvT_sb[64:128, :])

            # ascending other-half order means slots 12-15 shrink (512..128
            # cols), so the last-arriving x quarter carries the SMALLEST exps
            emit_S(12)
            emit_mask(12)
            emit_AV(4)
            emit_S(13)
            emit_mask(13)
            emit_AV(5)
            emit_S(14)
            emit_mask(14)
            emit_AV(6)
            emit_S(15)
            emit_mask(15)
            emit_AV(7)
            emit_AV(8)
            emit_AV(9)
            emit_AV(10)
            emit_AV(11)
            # o1 is complete at AV(11): drain it under the remaining tail
            nc.vector.tensor_copy(out=o_sb[:, 512:1024], in_=o_ps[1][:])
            nc.sync.dma_start(out=out_d[:, 512:1024], in_=o_sb[:, 512:1024])
            emit_AV(12)
            emit_AV(13)
            emit_AV(14)
            emit_AV(15)
            nc.scalar.activation(o_sb[:, 0:512], o_ps[0][:], Copy)
            nc.scalar.dma_start(out=out_d[:, 0:512], in_=o_sb[:, 0:512])

    nc.finalize()
    return nc


def _get_program():
    if "nc" not in _prog_cache:
        _prog_cache["nc"] = _build_program()
    return _prog_cache["nc"]


def make_in_maps(x, Wq, Wk, Wv):
    bf16 = ml_dtypes.bfloat16
    wcat = np.concatenate([Wq, Wk, Wv], axis=1).astype(bf16)  # [C, 192]
    tri = np.triu(np.ones((128, 128), np.float32))  # tri[k,q]=1 iff q>=k
    in_maps = []
    for core in range(8):
        b, r = core // 2, core % 2
        xb = np.asarray(x[b]).reshape(16, 128, C)
        own_blocks = [(14 + r) - 2 * i for i in range(8)]
        oth_blocks = [(1 - r) + 2 * m for m in range(8)]
        own = xb[own_blocks].reshape(TQ, C)
        other = xb[oth_blocks].reshape(TQ, C)
        xtT = np.concatenate([own, other], axis=0).T  # [C, 2048]
        # pre-tile for the DMA: row-block g=4*tb+u of the [2048,1024] DRAM
        # tensor holds (time-quarter tb) x (chunk pair 2u|2u+1)
        xt = (xtT.reshape(4, 2, 128, 4, 512)
              .transpose(3, 0, 2, 1, 4).reshape(NT, C))
        gate = np.full((128, 128), float(r), np.float32)
        trig = np.concatenate([tri, gate], axis=1).astype(bf16)  # [128, 256]
        in_maps.append({
            "xt": np.ascontiguousarray(xt).astype(bf16),
            "wcat": wcat,
            "trig": trig,
        })
    return in_maps


def postprocess(results):
    out = np.empty((B, T, H), np.float32)
    for core in range(8):
        b, r = core // 2, core % 2
        oT = np.asarray(results[core]["outT"], np.float32)  # [65, 1024]
        o = (oT[:H] / oT[H:H + 1]).T  # [1024, 64] local q order
        for i in range(8):
            g = (14 + r) - 2 * i
            out[b, 128 * g: 128 * (g + 1)] = o[128 * i: 128 * (i + 1)]
    return out


def kernel(x, mask, Wq, Wk, Wv, _trace=False, _tracedir=None):
    from concourse import bass_utils

    nc = _get_program()
    in_maps = make_in_maps(np.asarray(x, np.float32), np.asarray(Wq, np.float32),
                           np.asarray(Wk, np.float32), np.asarray(Wv, np.float32))
    res = bass_utils.run_bass_kernel_spmd(
        nc, in_maps, core_ids=list(range(8)),
        trace=_trace, tmpdir=_tracedir,
    )
    out = postprocess(res.results)
    if _trace:
        return out, res
    return out


# revision 70
# speedup vs baseline: 1.1177x; 1.0209x over previous
"""Trainium2 Bass kernel for single-head causal attention.

Problem: x[B=4,T=2048,C=1024] -> q,k,v = x@Wq/Wk/Wv [T,64] -> causal softmax(q k^T/sqrt(C)) @ v.

Sharding: 8 cores = 4 batches x 2 query-interleavings. Core r of a batch owns
the 8 INTERLEAVED query blocks g === r (mod 2) (128 rows each), which balances
causal work across the pair (each core gets ~half the attention area).

SPMD-uniform trick: the time axis of each core's x^T copy is permuted so the
core's OWN blocks come first in DESCENDING global order (columns 0-1023), the
other 8 blocks after (descending). Then the block-causal structure is
identical on every core and every S k-slot covers a PREFIX of the query axis:
  - k-slot t=0..7  (own block (14+r)-2t): S over q cols [0, 128(t+1)); the
    trailing 128x128 block is the diagonal -> multiplied by a triangular mask.
  - k-slot t=8..15 (other block (15-r)-2(t-8)): S over q cols [0, 128(t-7));
    the trailing block differs only by DATA: all-ones (r=1: k-block just
    below the diagonal -> keep) or all-zeros (r=0: just above -> drop).
Prefix ranges mean S/exp for slots 0..3 need only the first quarter of x, so
the scalar-engine exp pipeline (the S-phase bottleneck) starts while x is
still streaming in.

Softmax normalization is fused into the AV matmul by appending a ones column
to V (output row 64 = sum of exp); division happens host-side on gather.

All matmuls stream bf16. x is loaded as 16 separate 256KB DMAs (the ~0.65us
per-trigger cost paces the queue so transfers pipeline instead of splitting
HBM bandwidth round-robin). Projections run chunk-major per time-quarter. V
is transposed to natural layout with four batched DMA-xbar transposes.
"""

import numpy as np
import ml_dtypes

B, T, C, H = 4, 2048, 1024, 64
TQ = 1024          # queries per core
NT = 2048          # kv length per core
NCH = C // 128     # 8 contraction chunks
NKT = NT // 128    # 16 k-slots
SCALE = 1.0 / 32.0  # 1/sqrt(C)
VSTRIDE = 80       # bf16 cols per v' slot (64 v + 1 ones + pad, 32B-aligned)

_prog_cache = {}


def _build_program():
    import concourse.mybir as mybir
    from concourse import bacc
    from concourse.tile import TileContext

    fp32 = mybir.dt.float32
    bf16 = mybir.dt.bfloat16
    Exp = mybir.ActivationFunctionType.Exp
    Copy = mybir.ActivationFunctionType.Copy

    nc = bacc.Bacc("TRN2", target_bir_lowering=False, debug=False)

    # xt is pre-tiled host-side: row-block g=4*tb+u is one [128,1024] SBUF
    # tile (time-quarter tb, C-chunk pair 2u|2u+1), DRAM-contiguous.
    xt_d = nc.dram_tensor("xt", [NT, C], bf16, kind="ExternalInput")
    wcat_d = nc.dram_tensor("wcat", [C, 192], bf16, kind="ExternalInput")
    trig_d = nc.dram_tensor("trig", [128, 256], bf16, kind="ExternalInput")
    out_d = nc.dram_tensor("outT", [H + 1, TQ], bf16, kind="ExternalOutput")
    scr_d = nc.dram_tensor("scr", [64, 64], bf16, kind="Internal")

    with TileContext(nc) as tc:
        with (
            tc.tile_pool(name="xtp", bufs=1) as xt_pool,
            tc.tile_pool(name="cst", bufs=1) as cst,
            tc.tile_pool(name="prj", bufs=1) as prj,
            tc.tile_pool(name="ptp", bufs=16) as ptp,
            tc.tile_pool(name="pqk", bufs=2, space="PSUM") as pqk,
            tc.tile_pool(name="pvo", bufs=2, space="PSUM") as pvo,
            tc.tile_pool(name="pss", bufs=2, space="PSUM") as pss,
        ):
            # const DMAs on the scalar queue (idle until exp starts)
            wcat_sb = cst.tile([128, NCH, 192], bf16, tag="wcat")
            nc.scalar.dma_start(out=wcat_sb[:], in_=wcat_d.rearrange("(o p) m -> p o m", p=128))
            trig_sb = cst.tile([128, 256], bf16, tag="trig")
            nc.scalar.dma_start(out=trig_sb[:], in_=trig_d[:])

            # x^T time-quarters, 4 chunk-pair DMAs each, on the sync queue.
            # Tiles rotate through 4 pool slots: DMA g+4 WAR-waits until the
            # proj matmuls consumed tile g, capping in-flight transfers so
            # arrivals pipeline instead of splitting HBM bandwidth 8+ ways.
            xtiles = [xt_pool.tile([128, 1024], bf16, tag=f"x{g}", bufs=1,
                                   name=f"x{g}") for g in range(16)]
            xd = [xtiles[4 * tb: 4 * tb + 4] for tb in range(4)]
            for g in range(16):
                nc.sync.dma_start(out=xtiles[g][:],
                                  in_=xt_d[128 * g: 128 * (g + 1), :])

            def xch(tb, c):
                """rhs AP [128, 512] for time-quarter tb, contraction chunk c."""
                return xd[tb][c // 2][:, 512 * (c % 2): 512 * (c % 2) + 512]

            # persistent projection outputs (bf16 so S/AV stream at full rate)
            qT_sb = prj.tile([64, TQ], bf16, tag="qT")
            kT_sb = prj.tile([64, TQ], bf16, tag="kT")      # own-half k
            kvT_sb = prj.tile([128, TQ], bf16, tag="kvT")   # rows 0-63 k-oth, 64-127 v-oth
            vT_sb = prj.tile([64, TQ], bf16, tag="vT")      # v-own (transposed)
            vp_sb = prj.tile([128, NKT * VSTRIDE], bf16, tag="vp")
            o_sb = prj.tile([H + 1, TQ], bf16, tag="osb")

            vp3 = vp_sb.rearrange("p (t c) -> p t c", c=VSTRIDE)
            nc.gpsimd.memset(vp3[:, :, 64:65], 1.0)

            # ---- PE warmup on memset data: keep the HAM activity window
            # busy from engine-init through the first x arrivals ----
            ws_sb = prj.tile([128, 256], bf16, tag="ws")
            wsc_sb = prj.tile([64, 64], bf16, tag="wsc")
            nc.vector.memset(ws_sb[:], 0.125)
            w_ps = pss.tile([128, 256], fp32, tag="s")
            for _ in range(16):
                nc.tensor.matmul(w_ps[:], ws_sb[:, 0:128], ws_sb[:],
                                 start=True, stop=True)
            nc.vector.tensor_copy(out=wsc_sb[:], in_=w_ps[0:64, 0:64])

            # ---- projection passes (chunk-major within a time-quarter) ----
            def emit_proj(ps, lhs_lo, lhs_hi, tb):
                for c in range(NCH):
                    nc.tensor.matmul(
                        ps[:], wcat_sb[:, c, lhs_lo:lhs_hi], xch(tb, c),
                        start=(c == 0), stop=(c == NCH - 1),
                    )

            def emit_proj_qkv(qk_ps, v_ps, tb):
                # same-psum matmul runs back to back (bank-alternating costs
                # ~60ns/MM extra); v trails by a chunk so it never stalls
                for c in range(NCH):
                    nc.tensor.matmul(
                        qk_ps[:], wcat_sb[:, c, 0:128], xch(tb, c),
                        start=(c == 0), stop=(c == NCH - 1),
                    )
                for c in range(NCH):
                    nc.tensor.matmul(
                        v_ps[:], wcat_sb[:, c, 128:192], xch(tb, c),
                        start=(c == 0), stop=(c == NCH - 1),
                    )

            # ---- S / exp / AV phase ----
            pt_tiles = {}

            def emit_S(t):
                tp = t % 8 if t < 8 else 15 - t
                hi = 128 * (tp + 1)
                kt = (kT_sb[:, 128 * tp: 128 * (tp + 1)] if t < 8
                      else kvT_sb[0:64, 128 * (t - 8): 128 * (t - 7)])
                s = pss.tile([128, 1024], fp32, tag="s")
                nc.tensor.matmul(s[:, 0:min(hi, 512)], kt,
                                 qT_sb[:, 0:min(hi, 512)],
                                 start=True, stop=True)
                if hi > 512:
                    nc.tensor.matmul(s[:, 512:hi], kt, qT_sb[:, 512:hi],
                                     start=True, stop=True)
                pt = ptp.tile([128, 1024], bf16, tag="pt")
                nc.scalar.activation(pt[:, 0:hi], s[:, 0:hi], Exp, scale=SCALE)
                pt_tiles[t] = pt

            def emit_mask(t):
                tp = t % 8 if t < 8 else 15 - t
                hi = 128 * (tp + 1)
                msk = trig_sb[:, 0:128] if t < 8 else trig_sb[:, 128:256]
                pt = pt_tiles[t]
                nc.vector.tensor_mul(pt[:, 128 * tp: hi], pt[:, 128 * tp: hi], msk)

            o_ps = []

            def emit_AV(t):
                tp = t % 8 if t < 8 else 15 - t
                hi = 128 * (tp + 1)
                pt = pt_tiles.pop(t)
                nc.tensor.matmul(
                    o_ps[0][:, 0:min(hi, 512)],
                    vp3[:, t, 0:65], pt[:, 0:min(hi, 512)],
                    start=(t == 0), stop=(t == 15),
                    skip_group_check=True,
                )
                if hi > 512:
                    nc.tensor.matmul(
                        o_ps[1][:, 0:hi - 512],
                        vp3[:, t, 0:65], pt[:, 512:hi],
                        start=(t == 4), stop=(t == 11),
                        skip_group_check=True,
                    )

            # ================= program order =================
            # pass 1-A: own cols [0:512) -> q[0:512), k-own 0..3, v-own-A
            qk_psA = pqk.tile([128, 512], fp32, tag="qk")
            v_psA = pvo.tile([64, 512], fp32, tag="vo", padded_shape=[128, 512])
            emit_proj_qkv(qk_psA, v_psA, 0)
            # halves: S(0)/S(1) only need the first 256 cols of q,k
            nc.vector.tensor_copy(out=qT_sb[:, 0:256], in_=qk_psA[0:64, 0:256])
            nc.scalar.activation(kT_sb[:, 0:256], qk_psA[64:128, 0:256], Copy)
            nc.vector.tensor_copy(out=qT_sb[:, 256:512], in_=qk_psA[0:64, 256:512])
            nc.scalar.activation(kT_sb[:, 256:512], qk_psA[64:128, 256:512], Copy)
            nc.vector.tensor_copy(out=vT_sb[:, 0:512], in_=v_psA[:])

            emit_S(0)
            emit_S(1)
            emit_S(2)
            emit_S(3)
            emit_mask(0)
            emit_mask(1)
            emit_mask(2)
            emit_mask(3)

            # pass 1-B: own cols [512:1024)
            qk_psB = pqk.tile([128, 512], fp32, tag="qk")
            v_psB = pvo.tile([64, 512], fp32, tag="vo", padded_shape=[128, 512])
            emit_proj_qkv(qk_psB, v_psB, 1)
            nc.vector.tensor_copy(out=qT_sb[:, 512:1024], in_=qk_psB[0:64, :])
            nc.scalar.activation(kT_sb[:, 512:1024], qk_psB[64:128, :], Copy)
            nc.vector.tensor_copy(out=vT_sb[:, 512:1024], in_=v_psB[:])

            o_ps.append(pvo.tile([H + 1, 512], fp32, tag="vo", name="o0",
                                 padded_shape=[128, 512]))
            o_ps.append(pvo.tile([H + 1, 512], fp32, tag="vo", name="o1",
                                 padded_shape=[128, 512]))

            # all own-half v -> natural layout, one batched xbar transpose
            nc.sync.dma_start_transpose(out=vp3[:, 0:8, 0:64], in_=vT_sb[:])

            emit_S(4)
            emit_S(5)
            emit_S(6)
            emit_S(7)

            # pass 2-A: other cols [0:512) -> k-oth 0..3 + v-oth (packed)
            kv_psA = pqk.tile([128, 512], fp32, tag="qk")
            emit_proj(kv_psA, 64, 192, 2)
            nc.vector.tensor_copy(out=kvT_sb[:, 0:256], in_=kv_psA[:, 0:256])
            nc.vector.tensor_copy(out=kvT_sb[:, 256:512], in_=kv_psA[:, 256:512])

            emit_mask(4)
            emit_mask(5)
            emit_mask(6)
            emit_mask(7)
            emit_S(8)
            emit_mask(8)
            emit_S(9)
            emit_mask(9)
            emit_S(10)
            emit_mask(10)
            emit_S(11)
            emit_mask(11)
            emit_AV(0)
            emit_AV(1)
            emit_AV(2)
            emit_AV(3)

            # pass 2-B: other cols [512:1024)
            kv_psB = pqk.tile([128, 512], fp32, tag="qk")
            emit_proj(kv_psB, 64, 192, 3)
            nc.vector.tensor_copy(out=kvT_sb[:, 512:768], in_=kv_psB[:, 0:256])
            nc.vector.tensor_copy(out=kvT_sb[:, 768:1024], in_=kv_psB[:, 256:512])
            # all other-half v -> natural layout
            nc.sync.dma_start_transpose(out=vp3[:, 8:16, 0:64],
                                        in_=kvT_sb[64:128, :])

            # ascending other-half order means slots 12-15 shrink (512..128
            # cols), so the last-arriving x quarter carries the SMALLEST exps
            emit_S(12)
            emit_mask(12)
            emit_AV(4)
            emit_S(13)
            emit_mask(13)
            emit_AV(5)
            emit_S(14)
            emit_mask(14)
            emit_AV(6)
            emit_S(15)
            emit_mask(15)
            emit_AV(7)
            emit_AV(8)
            emit_AV(9)
            emit_AV(10)
            emit_AV(11)
            # o1 is complete at AV(11): drain it under the remaining tail
            nc.vector.tensor_copy(out=o_sb[:, 512:1024], in_=o_ps[1][:])
            nc.sync.dma_start(out=out_d[:, 512:1024], in_=o_sb[:, 512:1024])
            emit_AV(12)
            emit_AV(13)
            emit_AV(14)
            emit_AV(15)
            nc.scalar.activation(o_sb[:, 0:512], o_ps[0][:], Copy)
            nc.scalar.dma_start(out=out_d[:, 0:512], in_=o_sb[:, 0:512])

    nc.finalize()
    return nc


def _get_program():
    if "nc" not in _prog_cache:
        _prog_cache["nc"] = _build_program()
    return _prog_cache["nc"]


def make_in_maps(x, Wq, Wk, Wv):
    bf16 = ml_dtypes.bfloat16
    wcat = np.concatenate([Wq, Wk, Wv], axis=1).astype(bf16)  # [C, 192]
    tri = np.triu(np.ones((128, 128), np.float32))  # tri[k,q]=1 iff q>=k
    in_maps = []
    for core in range(8):
        b, r = core // 2, core % 2
        xb = np.asarray(x[b]).reshape(16, 128, C)
        own_blocks = [(14 + r) - 2 * i for i in range(8)]
        oth_blocks = [(1 - r) + 2 * m for m in range(8)]
        own = xb[own_blocks].reshape(TQ, C)
        other = xb[oth_blocks].reshape(TQ, C)
        xtT = np.concatenate([own, other], axis=0).T  # [C, 2048]
        # pre-tile for the DMA: row-block g=4*tb+u of the [2048,1024] DRAM
        # tensor holds (time-quarter tb) x (chunk pair 2u|2u+1)
        xt = (xtT.reshape(4, 2, 128, 4, 512)
              .transpose(3, 0, 2, 1, 4).reshape(NT, C))
        gate = np.full((128, 128), float(r), np.float32)
        trig = np.concatenate([tri, gate], axis=1).astype(bf16)  # [128, 256]
        in_maps.append({
            "xt": np.ascontiguousarray(xt).astype(bf16),
            "wcat": wcat,
            "trig": trig,
        })
    return in_maps


def postprocess(results):
    out = np.empty((B, T, H), np.float32)
    for core in range(8):
        b, r = core // 2, core % 2
        oT = np.asarray(results[core]["outT"], np.float32)  # [65, 1024]
        o = (oT[:H] / oT[H:H + 1]).T  # [1024, 64] local q order
        for i in range(8):
            g = (14 + r) - 2 * i
            out[b, 128 * g: 128 * (g + 1)] = o[128 * i: 128 * (i + 1)]
    return out


def kernel(x, mask, Wq, Wk, Wv, _trace=False, _tracedir=None):
    from concourse import bass_utils

    nc = _get_program()
    in_maps = make_in_maps(np.asarray(x, np.float32), np.asarray(Wq, np.float32),
                           np.asarray(Wk, np.float32), np.asarray(Wv, np.float32))
    res = bass_utils.run_bass_kernel_spmd(
        nc, in_maps, core_ids=list(range(8)),
        trace=_trace, tmpdir=_tracedir,
    )
    out = postprocess(res.results)
    if _trace:
        return out, res
    return out


# revision 71
# speedup vs baseline: 1.1317x; 1.0126x over previous
"""Trainium2 Bass kernel for single-head causal attention.

Problem: x[B=4,T=2048,C=1024] -> q,k,v = x@Wq/Wk/Wv [T,64] -> causal softmax(q k^T/sqrt(C)) @ v.

Sharding: 8 cores = 4 batches x 2 query-interleavings. Core r of a batch owns
the 8 INTERLEAVED query blocks g === r (mod 2) (128 rows each), which balances
causal work across the pair (each core gets ~half the attention area).

SPMD-uniform trick: the time axis of each core's x^T copy is permuted so the
core's OWN blocks come first in DESCENDING global order (columns 0-1023), the
other 8 blocks after (descending). Then the block-causal structure is
identical on every core and every S k-slot covers a PREFIX of the query axis:
  - k-slot t=0..7  (own block (14+r)-2t): S over q cols [0, 128(t+1)); the
    trailing 128x128 block is the diagonal -> multiplied by a triangular mask.
  - k-slot t=8..15 (other block (15-r)-2(t-8)): S over q cols [0, 128(t-7));
    the trailing block differs only by DATA: all-ones (r=1: k-block just
    below the diagonal -> keep) or all-zeros (r=0: just above -> drop).
Prefix ranges mean S/exp for slots 0..3 need only the first quarter of x, so
the scalar-engine exp pipeline (the S-phase bottleneck) starts while x is
still streaming in.

Softmax normalization is fused into the AV matmul by appending a ones column
to V (output row 64 = sum of exp); division happens host-side on gather.

All matmuls stream bf16. x is loaded as 16 separate 256KB DMAs (the ~0.65us
per-trigger cost paces the queue so transfers pipeline instead of splitting
HBM bandwidth round-robin). Projections run chunk-major per time-quarter. V
is transposed to natural layout with four batched DMA-xbar transposes.
"""

import numpy as np
import ml_dtypes

B, T, C, H = 4, 2048, 1024, 64
TQ = 1024          # queries per core
NT = 2048          # kv length per core
NCH = C // 128     # 8 contraction chunks
NKT = NT // 128    # 16 k-slots
SCALE = 1.0 / 32.0  # 1/sqrt(C)
VSTRIDE = 80       # bf16 cols per v' slot (64 v + 1 ones + pad, 32B-aligned)

_prog_cache = {}


def _build_program():
    import concourse.mybir as mybir
    from concourse import bacc
    from concourse.tile import TileContext

    fp32 = mybir.dt.float32
    bf16 = mybir.dt.bfloat16
    Exp = mybir.ActivationFunctionType.Exp
    Copy = mybir.ActivationFunctionType.Copy

    nc = bacc.Bacc("TRN2", target_bir_lowering=False, debug=False)

    # xt is pre-tiled host-side: row-block g=4*tb+u is one [128,1024] SBUF
    # tile (time-quarter tb, C-chunk pair 2u|2u+1), DRAM-contiguous.
    xt_d = nc.dram_tensor("xt", [NT, C], bf16, kind="ExternalInput")
    wcat_d = nc.dram_tensor("wcat", [C, 192], bf16, kind="ExternalInput")
    trig_d = nc.dram_tensor("trig", [128, 256], bf16, kind="ExternalInput")
    out_d = nc.dram_tensor("outT", [H + 1, TQ], bf16, kind="ExternalOutput")
    scr_d = nc.dram_tensor("scr", [64, 64], bf16, kind="Internal")

    with TileContext(nc) as tc:
        with (
            tc.tile_pool(name="xtp", bufs=1) as xt_pool,
            tc.tile_pool(name="cst", bufs=1) as cst,
            tc.tile_pool(name="prj", bufs=1) as prj,
            tc.tile_pool(name="ptp", bufs=16) as ptp,
            tc.tile_pool(name="pqk", bufs=2, space="PSUM") as pqk,
            tc.tile_pool(name="pvo", bufs=2, space="PSUM") as pvo,
            tc.tile_pool(name="pss", bufs=2, space="PSUM") as pss,
        ):
            # const DMAs on the scalar queue (idle until exp starts)
            wcat_sb = cst.tile([128, NCH, 192], bf16, tag="wcat")
            nc.scalar.dma_start(out=wcat_sb[:], in_=wcat_d.rearrange("(o p) m -> p o m", p=128))
            trig_sb = cst.tile([128, 256], bf16, tag="trig")
            nc.scalar.dma_start(out=trig_sb[:], in_=trig_d[:])

            # x^T time-quarters, 4 chunk-pair DMAs each, on the sync queue.
            # Tiles rotate through 4 pool slots: DMA g+4 WAR-waits until the
            # proj matmuls consumed tile g, capping in-flight transfers so
            # arrivals pipeline instead of splitting HBM bandwidth 8+ ways.
            xtiles = [xt_pool.tile([128, 1024], bf16, tag=f"x{g}", bufs=1,
                                   name=f"x{g}") for g in range(16)]
            xd = [xtiles[4 * tb: 4 * tb + 4] for tb in range(4)]
            for g in range(16):
                nc.sync.dma_start(out=xtiles[g][:],
                                  in_=xt_d[128 * g: 128 * (g + 1), :])

            def xch(tb, c):
                """rhs AP [128, 512] for time-quarter tb, contraction chunk c."""
                return xd[tb][c // 2][:, 512 * (c % 2): 512 * (c % 2) + 512]

            # persistent projection outputs (bf16 so S/AV stream at full rate)
            qT_sb = prj.tile([64, TQ], bf16, tag="qT")
            kT_sb = prj.tile([64, TQ], bf16, tag="kT")      # own-half k
            kvT_sb = prj.tile([128, TQ], bf16, tag="kvT")   # rows 0-63 k-oth, 64-127 v-oth
            vT_sb = prj.tile([64, TQ], bf16, tag="vT")      # v-own (transposed)
            vp_sb = prj.tile([128, NKT * VSTRIDE], bf16, tag="vp")
            o_sb = prj.tile([H + 1, TQ], bf16, tag="osb")

            vp3 = vp_sb.rearrange("p (t c) -> p t c", c=VSTRIDE)
            nc.gpsimd.memset(vp3[:, :, 64:65], 1.0)

            # ---- PE warmup on memset data: keep the HAM activity window
            # busy from engine-init through the first x arrivals ----
            ws_sb = prj.tile([128, 256], bf16, tag="ws")
            wsc_sb = prj.tile([64, 64], bf16, tag="wsc")
            nc.vector.memset(ws_sb[:], 0.125)
            w_ps = pss.tile([128, 256], fp32, tag="s")
            for _ in range(16):
                nc.tensor.matmul(w_ps[:], ws_sb[:, 0:128], ws_sb[:],
                                 start=True, stop=True)
            nc.vector.tensor_copy(out=wsc_sb[:], in_=w_ps[0:64, 0:64])

            # ---- projection passes (chunk-major within a time-quarter) ----
            def emit_proj(ps, lhs_lo, lhs_hi, tb):
                for c in range(NCH):
                    nc.tensor.matmul(
                        ps[:], wcat_sb[:, c, lhs_lo:lhs_hi], xch(tb, c),
                        start=(c == 0), stop=(c == NCH - 1),
                    )

            def emit_proj_qkv(qk_ps, v_ps, tb):
                # same-psum matmul runs back to back (bank-alternating costs
                # ~60ns/MM extra); v trails by a chunk so it never stalls
                for c in range(NCH):
                    nc.tensor.matmul(
                        qk_ps[:], wcat_sb[:, c, 0:128], xch(tb, c),
                        start=(c == 0), stop=(c == NCH - 1),
                    )
                for c in range(NCH):
                    nc.tensor.matmul(
                        v_ps[:], wcat_sb[:, c, 128:192], xch(tb, c),
                        start=(c == 0), stop=(c == NCH - 1),
                    )

            # ---- S / exp / AV phase ----
            pt_tiles = {}

            def emit_S(t):
                tp = t % 8 if t < 8 else 15 - t
                hi = 128 * (tp + 1)
                kt = (kT_sb[:, 128 * tp: 128 * (tp + 1)] if t < 8
                      else kvT_sb[0:64, 128 * (t - 8): 128 * (t - 7)])
                s = pss.tile([128, 1024], fp32, tag="s")
                nc.tensor.matmul(s[:, 0:min(hi, 512)], kt,
                                 qT_sb[:, 0:min(hi, 512)],
                                 start=True, stop=True)
                if hi > 512:
                    nc.tensor.matmul(s[:, 512:hi], kt, qT_sb[:, 512:hi],
                                     start=True, stop=True)
                pt = ptp.tile([128, 1024], bf16, tag="pt")
                nc.scalar.activation(pt[:, 0:hi], s[:, 0:hi], Exp, scale=SCALE)
                pt_tiles[t] = pt

            def emit_mask(t):
                tp = t % 8 if t < 8 else 15 - t
                hi = 128 * (tp + 1)
                msk = trig_sb[:, 0:128] if t < 8 else trig_sb[:, 128:256]
                pt = pt_tiles[t]
                nc.vector.tensor_mul(pt[:, 128 * tp: hi], pt[:, 128 * tp: hi], msk)

            o_ps = []

            def emit_AV(t):
                tp = t % 8 if t < 8 else 15 - t
                hi = 128 * (tp + 1)
                pt = pt_tiles.pop(t)
                nc.tensor.matmul(
                    o_ps[0][:, 0:min(hi, 512)],
                    vp3[:, t, 0:65], pt[:, 0:min(hi, 512)],
                    start=(t == 0), stop=(t == 15),
                    skip_group_check=True,
                )
                if hi > 512:
                    nc.tensor.matmul(
                        o_ps[1][:, 0:hi - 512],
                        vp3[:, t, 0:65], pt[:, 512:hi],
                        start=(t == 4), stop=(t == 11),
                        skip_group_check=True,
                    )

            # ================= program order =================
            # pass 1-A: own cols [0:512) -> q[0:512), k-own 0..3, v-own-A
            qk_psA = pqk.tile([128, 512], fp32, tag="qk")
            v_psA = pvo.tile([64, 512], fp32, tag="vo", padded_shape=[128, 512])
            emit_proj_qkv(qk_psA, v_psA, 0)
            # halves: S(0)/S(1) only need the first 256 cols of q,k
            nc.vector.tensor_copy(out=qT_sb[:, 0:256], in_=qk_psA[0:64, 0:256])
            nc.scalar.activation(kT_sb[:, 0:256], qk_psA[64:128, 0:256], Copy)
            nc.vector.tensor_copy(out=qT_sb[:, 256:512], in_=qk_psA[0:64, 256:512])
            nc.scalar.activation(kT_sb[:, 256:512], qk_psA[64:128, 256:512], Copy)
            nc.vector.tensor_copy(out=vT_sb[:, 0:512], in_=v_psA[:])

            emit_S(0)
            emit_S(1)
            emit_S(2)
            emit_S(3)
            emit_mask(0)
            emit_mask(1)
            emit_mask(2)
            emit_mask(3)

            # pass 1-B: own cols [512:1024)
            qk_psB = pqk.tile([128, 512], fp32, tag="qk")
            v_psB = pvo.tile([64, 512], fp32, tag="vo", padded_shape=[128, 512])
            emit_proj_qkv(qk_psB, v_psB, 1)
            nc.vector.tensor_copy(out=qT_sb[:, 512:768], in_=qk_psB[0:64, 0:256])
            nc.scalar.activation(kT_sb[:, 512:768], qk_psB[64:128, 0:256], Copy)
            nc.vector.tensor_copy(out=qT_sb[:, 768:1024], in_=qk_psB[0:64, 256:512])
            nc.scalar.activation(kT_sb[:, 768:1024], qk_psB[64:128, 256:512], Copy)
            nc.vector.tensor_copy(out=vT_sb[:, 512:1024], in_=v_psB[:])

            o_ps.append(pvo.tile([H + 1, 512], fp32, tag="vo", name="o0",
                                 padded_shape=[128, 512]))
            o_ps.append(pvo.tile([H + 1, 512], fp32, tag="vo", name="o1",
                                 padded_shape=[128, 512]))

            # all own-half v -> natural layout, one batched xbar transpose
            nc.sync.dma_start_transpose(out=vp3[:, 0:8, 0:64], in_=vT_sb[:])

            emit_S(4)
            emit_S(5)
            emit_S(6)
            emit_S(7)

            # pass 2-A: other cols [0:512) -> k-oth 0..3 + v-oth (packed)
            kv_psA = pqk.tile([128, 512], fp32, tag="qk")
            emit_proj(kv_psA, 64, 192, 2)
            nc.vector.tensor_copy(out=kvT_sb[:, 0:256], in_=kv_psA[:, 0:256])
            nc.vector.tensor_copy(out=kvT_sb[:, 256:512], in_=kv_psA[:, 256:512])

            emit_mask(4)
            emit_mask(5)
            emit_mask(6)
            emit_mask(7)
            emit_S(8)
            emit_mask(8)
            emit_S(9)
            emit_mask(9)
            emit_S(10)
            emit_mask(10)
            emit_S(11)
            emit_mask(11)
            emit_AV(0)
            emit_AV(1)
            emit_AV(2)
            emit_AV(3)

            # pass 2-B: other cols [512:1024)
            kv_psB = pqk.tile([128, 512], fp32, tag="qk")
            emit_proj(kv_psB, 64, 192, 3)
            nc.vector.tensor_copy(out=kvT_sb[:, 512:768], in_=kv_psB[:, 0:256])
            nc.vector.tensor_copy(out=kvT_sb[:, 768:1024], in_=kv_psB[:, 256:512])
            # all other-half v -> natural layout
            nc.sync.dma_start_transpose(out=vp3[:, 8:16, 0:64],
                                        in_=kvT_sb[64:128, :])

            # ascending other-half order means slots 12-15 shrink (512..128
            # cols), so the last-arriving x quarter carries the SMALLEST exps
            emit_S(12)
            emit_mask(12)
            emit_AV(4)
            emit_S(13)
            emit_mask(13)
            emit_AV(5)
            emit_S(14)
            emit_mask(14)
            emit_AV(6)
            emit_S(15)
            emit_mask(15)
            emit_AV(7)
            emit_AV(8)
            emit_AV(9)
            emit_AV(10)
            emit_AV(11)
            # o1 is complete at AV(11): drain it under the remaining tail
            nc.vector.tensor_copy(out=o_sb[:, 512:1024], in_=o_ps[1][:])
            nc.sync.dma_start(out=out_d[:, 512:1024], in_=o_sb[:, 512:1024])
            emit_AV(12)
            emit_AV(13)
            emit_AV(14)
            emit_AV(15)
            nc.scalar.activation(o_sb[:, 0:512], o_ps[0][:], Copy)
            nc.scalar.dma_start(out=out_d[:, 0:512], in_=o_sb[:, 0:512])

    nc.finalize()
    return nc


def _get_program():
    if "nc" not in _prog_cache:
        _prog_cache["nc"] = _build_program()
    return _prog_cache["nc"]


def make_in_maps(x, Wq, Wk, Wv):
    bf16 = ml_dtypes.bfloat16
    wcat = np.concatenate([Wq, Wk, Wv], axis=1).astype(bf16)  # [C, 192]
    tri = np.triu(np.ones((128, 128), np.float32))  # tri[k,q]=1 iff q>=k
    in_maps = []
    for core in range(8):
        b, r = core // 2, core % 2
        xb = np.asarray(x[b]).reshape(16, 128, C)
        own_blocks = [(14 + r) - 2 * i for i in range(8)]
        oth_blocks = [(1 - r) + 2 * m for m in range(8)]
        own = xb[own_blocks].reshape(TQ, C)
        other = xb[oth_blocks].reshape(TQ, C)
        xtT = np.concatenate([own, other], axis=0).T  # [C, 2048]
        # pre-tile for the DMA: row-block g=4*tb+u of the [2048,1024] DRAM
        # tensor holds (time-quarter tb) x (chunk pair 2u|2u+1)
        xt = (xtT.reshape(4, 2, 128, 4, 512)
              .transpose(3, 0, 2, 1, 4).reshape(NT, C))
        gate = np.full((128, 128), float(r), np.float32)
        trig = np.concatenate([tri, gate], axis=1).astype(bf16)  # [128, 256]
        in_maps.append({
            "xt": np.ascontiguousarray(xt).astype(bf16),
            "wcat": wcat,
            "trig": trig,
        })
    return in_maps


def postprocess(results):
    out = np.empty((B, T, H), np.float32)
    for core in range(8):
        b, r = core // 2, core % 2
        oT = np.asarray(results[core]["outT"], np.float32)  # [65, 1024]
        o = (oT[:H] / oT[H:H + 1]).T  # [1024, 64] local q order
        for i in range(8):
            g = (14 + r) - 2 * i
            out[b, 128 * g: 128 * (g + 1)] = o[128 * i: 128 * (i + 1)]
    return out


def kernel(x, mask, Wq, Wk, Wv, _trace=False, _tracedir=None):
    from concourse import bass_utils

    nc = _get_program()
    in_maps = make_in_maps(np.asarray(x, np.float32), np.asarray(Wq, np.float32),
                           np.asarray(Wk, np.float32), np.asarray(Wv, np.float32))
    res = bass_utils.run_bass_kernel_spmd(
        nc, in_maps, core_ids=list(range(8)),
        trace=_trace, tmpdir=_tracedir,
    )
    out = postprocess(res.results)
    if _trace:
        return out, res
    return out
